# revision 47
# baseline (speedup 1.0000x reference)
"""Trainium2 raw-Bass kernel for nn_BasicRNNBlock (vanilla tanh RNN).

Reference semantics (fp32):
    xp = einsum("bti,hi->tbh", x, W_ih) + b_ih + b_hh      # input projection
    h_t = tanh(xp_t + h_{t-1} @ W_hh.T),  h_0 = 0          # T sequential steps
    out[b, t, :] = h_t[b]                                  # [B, T, H]

Shapes: B=64, T=512, I=H=1024.  Sharding: data-parallel over batch across
8 NeuronCores (8 batches/core, weights replicated).  fp16 matmul inputs,
fp32 PSUM accumulation.

This version is written in raw Bass (manual semaphores) instead of the Tile
framework.  Tile attached a semaphore update to every matmul and its
conservative scheduling left ~500ns/step of PE stalls at the tanh
dependency; here only 2 of the 66 PE instructions per step carry an update
and the ordering hides the scalar-engine latency entirely.  The steady
state runs at the LDWEIGHTS-throughput floor (~27ns per 128x128 W_hh block
pair, 64 blocks/step) plus one interleaved 512-col projection matmul.

Per-core structure per step t (PE queue order):
  inject:  2 identity matmuls write xp_t into the lo/hi PSUM banks
  proj:    one N=512 projection matmul for slice t//64+1 (its streaming
           time absorbs the tanh_lo(t-1) latency)
  lo phase: k=0..3 x c=0..7 W_hh-block matmuls (needs tanh_lo(t-1))
  hi phase: k=4..7, c=0..3 then c=4..7 (needs tanh_hi(t-1)); sem incs on
            the last lo-complete and last hi-complete matmuls
ACT: tanh_lo / tanh_hi into a 16-step output slab (SBUF) that doubles as
     the h source for the next step's matmuls; slabs DMA to HBM every 16
     steps.  tanh_lo/tanh_hi read separate PSUM banks (3-deep rings each)
     because a ScalarE read of a bank TensorE is still writing is a fatal
     PSUM collision.
"""
import numpy as np

B, T, I, H = 64, 512, 1024, 1024
N_CORES = 8
BS = B // N_CORES          # 8 batches per core
GY = 16                    # steps per output slab
WIN = 64                   # steps per projection slice


def _build_program(steps=T):
    from contextlib import ExitStack

    from concourse import bacc, mybir

    f16 = mybir.dt.float16
    f32 = mybir.dt.float32
    Tanh = mybir.ActivationFunctionType.Tanh

    assert steps % WIN == 0
    n_slices = steps // WIN
    n_slabs = steps // GY

    nc = bacc.Bacc(None, target_bir_lowering=False)

    wih = nc.declare_dram_parameter("wih", [128, 8192], f16, isOutput=False)
    whh = nc.declare_dram_parameter("whh", [128, 8192], f16, isOutput=False)
    xt = nc.declare_dram_parameter("xt", [128, n_slices * 8 * 512], f16,
                                   isOutput=False)
    ident = nc.declare_dram_parameter("ident", [128, 128], f16, isOutput=False)
    bias = nc.declare_dram_parameter("bias", [128, 8], f32, isOutput=False)
    y = nc.declare_dram_parameter("y", [n_slabs, 128, GY * 64], f16,
                                  isOutput=True)

    with ExitStack() as ctx:
        ec = ctx.enter_context
        whh_sb = ec(nc.sbuf_tensor("whh_sb", [128, 8192], f16))
        wih_sb = ec(nc.sbuf_tensor("wih_sb", [128, 8192], f16))
        ident_sb = ec(nc.sbuf_tensor("ident_sb", [128, 128], f16))
        bias_sb = ec(nc.sbuf_tensor("bias_sb", [128, 8], f32))
        xsl = [ec(nc.sbuf_tensor(f"xsl{i}", [128, 8 * 512], f16))
               for i in range(2)]
        # all of xp stays resident in SBUF (64KB/partition at 512 steps)
        xp_all = ec(nc.sbuf_tensor("xp_all", [128, n_slices * 8 * 512], f16))
        yslab = [ec(nc.sbuf_tensor(f"yslab{i}", [128, GY * 64], f16))
                 for i in range(2)]
        # PSUM collisions are fatal at bank granularity: ScalarE may not read
        # a bank TensorE is still writing.  tanh_lo reads while the hi-half
        # matmuls run, so lo and hi live in separate banks (3-deep rings).
        rp_lo = [ec(nc.psum_tensor(f"rplo{i}", [128, 32], f32)) for i in range(3)]
        rp_hi = [ec(nc.psum_tensor(f"rphi{i}", [128, 32], f32)) for i in range(3)]
        pp = [ec(nc.psum_tensor(f"pp{i}", [128, 512], f32)) for i in range(2)]

        wih_sem = nc.alloc_semaphore("wih_sem")  # wih DMA (gates first proj)
        w_sem = nc.alloc_semaphore("w_sem")      # weight/const DMA loads
        in_sem = nc.alloc_semaphore("in_sem")    # xsl chunk DMA loads
        y_sem = nc.alloc_semaphore("y_sem")      # output slab DMAs
        pe_sem = nc.alloc_semaphore("pe_sem")    # PE progress: +2 per step
        act_sem = nc.alloc_semaphore("act_sem")  # ACT progress: +2 per step
        dve_sem = nc.alloc_semaphore("dve_sem")  # DVE bias-adds: +1 per chunk
        proj_sem = nc.alloc_semaphore("proj_sem")  # proj chunk: +1 per chunk

        # ---------------- SYNC engine: const loads + y slab DMAs ----------
        # wih first on its own sem: phase 1 only needs wih + xsl slice 0.
        # Four separate dma_starts spread across DMA engines (a single 2MB
        # dma_start sustains only ~100GB/s and cost ~18us of lead-in).
        for qi in range(3):
            nc.sync.dma_start(
                wih_sb[:, qi * 2048:(qi + 1) * 2048],
                wih[:, qi * 2048:(qi + 1) * 2048],
            ).then_inc(wih_sem, 16)
        nc.sync.dma_start(bias_sb[:, :], bias[:, :]).then_inc(w_sem, 16)
        nc.sync.dma_start(ident_sb[:, :], ident[:, :]).then_inc(w_sem, 16)
        nc.sync.dma_start(whh_sb[:, :], whh[:, :]).then_inc(w_sem, 16)
        for g in range(n_slabs):
            nc.sync.wait_ge(act_sem, 2 * (GY * g + GY))
            if g > 0:
                # serialize slab-DMA completions so y_sem counts in order
                nc.sync.wait_ge(y_sem, 16 * g)
            nc.sync.dma_start(y[g], yslab[g % 2][:, :]).then_inc(y_sem, 16)
        nc.sync.wait_ge(y_sem, 16 * n_slabs)

        # ---------------- GPSIMD engine: xsl slice loads ------------------
        def load_slice(q):
            buf = xsl[q % 2]
            for k in range(8):
                nc.gpsimd.dma_start(
                    buf[:, k * 512:(k + 1) * 512],
                    xt[:, k * (n_slices * 512) + q * 512:
                       k * (n_slices * 512) + (q + 1) * 512],
                ).then_inc(in_sem, 16)

        load_slice(0)
        nc.gpsimd.dma_start(
            wih_sb[:, 6144:8192], wih[:, 6144:8192]).then_inc(wih_sem, 16)
        if n_slices > 1:
            load_slice(1)
        for s in range(n_slices):
            if s + 2 < n_slices:
                # buffer (s+2)%2 == s%2 was last read by the projection
                # matmuls for slice s (issued during slice s-1)
                nc.gpsimd.wait_ge(proj_sem, 8 * s + 8)
                load_slice(s + 2)

        # ---------------- PE engine ---------------------------------------
        def proj_mm(s, cp, kp):
            """One projection matmul: slice s, output chunk cp, k-chunk kp."""
            if kp == 0:
                if cp == 0:
                    # xsl buffer for slice s fully loaded
                    nc.tensor.wait_ge(in_sem, 16 * 8 * (s + 1))
                # pp bank cp%2 last read by DVE add (s, cp-2)
                v = 8 * s + cp - 1
                if v > 0:
                    nc.tensor.wait_ge(dve_sem, v)
            mm = nc.tensor.matmul(
                pp[cp % 2][:, :],
                wih_sb[:, kp * 1024 + cp * 128: kp * 1024 + (cp + 1) * 128],
                xsl[s % 2][:, kp * 512:(kp + 1) * 512],
                start=(kp == 0), stop=(kp == 7), skip_group_check=True,
            )
            if kp == 7:
                mm.then_inc(proj_sem, 1)

        # phase 1: the whole input projection as one dense block of 512-col
        # matmuls (back-to-back streaming keeps the HAM clock governor at
        # K=8/8, ~2x the throttled rate the interleaved version ran at)
        nc.tensor.wait_ge(wih_sem, 64)
        for s in range(n_slices):
            for cp in range(8):
                for kp in range(8):
                    proj_mm(s, cp, kp)

        # recurrence needs ident + whh (+ bias via DVE)
        nc.tensor.wait_ge(w_sem, 48)

        # phase 2: pure recurrence (no interleaved projection work)
        for t in range(steps):
            s, local = divmod(t, WIN)
            g, idx = divmod(t, GY)
            lo, hi = rp_lo[t % 3], rp_hi[t % 3]

            # inject xp_t into this step's PSUM slots (lo then hi)
            if local == 0:
                nc.tensor.wait_ge(dve_sem, 8 * s + 8)
            if t >= 3:
                nc.tensor.wait_ge(act_sem, 2 * t - 4)
            xp3 = xp_all[:, s * 4096:(s + 1) * 4096].rearrange(
                "p (c n) -> p c n", c=8)
            inj_lo = nc.tensor.matmul(
                lo[:, :].rearrange("p (c n) -> p c n", c=4),
                ident_sb[:, :],
                xp3[:, 0:4, local * 8:(local + 1) * 8],
                start=True, stop=(t == 0), skip_group_check=True,
            )
            inj_hi = nc.tensor.matmul(
                hi[:, :].rearrange("p (c n) -> p c n", c=4),
                ident_sb[:, :],
                xp3[:, 4:8, local * 8:(local + 1) * 8],
                start=True, stop=(t == 0), skip_group_check=True,
            )
            if t == 0:
                inj_lo.then_inc(pe_sem, 1)
                inj_hi.then_inc(pe_sem, 1)
            else:
                hb = yslab[((t - 1) // GY) % 2]
                hoff = ((t - 1) % GY) * 64

                def wmm(c, k, stop=False):
                    dst = lo if c < 4 else hi
                    return nc.tensor.matmul(
                        dst[:, (c % 4) * 8:(c % 4 + 1) * 8],
                        whh_sb[:, k * 1024 + c * 128: k * 1024 + (c + 1) * 128],
                        hb[:, hoff + k * 8: hoff + (k + 1) * 8],
                        start=False, stop=stop, skip_group_check=True,
                    )

                # lo phase: k 0..3 (needs tanh_lo(t-1))
                nc.tensor.wait_ge(act_sem, 2 * t - 1)
                for k in range(4):
                    for c in range(8):
                        wmm(c, k)
                # hi phase: k 4..7 (needs tanh_hi(t-1)); lo bank finishes first
                nc.tensor.wait_ge(act_sem, 2 * t)
                for k in range(4, 8):
                    for c in range(4):
                        mm = wmm(c, k, stop=(k == 7 and c == 3))
                        if k == 7 and c == 3:
                            mm.then_inc(pe_sem, 1)
                for k in range(4, 8):
                    for c in range(4, 8):
                        mm = wmm(c, k, stop=(k == 7 and c == 7))
                        if k == 7 and c == 7:
                            mm.then_inc(pe_sem, 1)

        # ---------------- ACT engine: tanh lo/hi per step ------------------
        for t in range(steps):
            g, idx = divmod(t, GY)
            out = yslab[g % 2]
            if idx == 0 and g >= 2:
                nc.scalar.wait_ge(y_sem, 16 * (g - 1))
            nc.scalar.wait_ge(pe_sem, 2 * t + 1)
            nc.scalar.activation(
                out[:, idx * 64: idx * 64 + 32], rp_lo[t % 3][:, :], Tanh,
            ).then_inc(act_sem, 1)
            nc.scalar.wait_ge(pe_sem, 2 * t + 2)
            nc.scalar.activation(
                out[:, idx * 64 + 32: idx * 64 + 64], rp_hi[t % 3][:, :], Tanh,
            ).then_inc(act_sem, 1)

        # ---------------- DVE engine: bias add, PSUM -> xp SBUF ------------
        first = True
        for s in range(n_slices):
            for cp in range(8):
                nc.vector.wait_ge(proj_sem, 8 * s + cp + 1)
                if first:
                    nc.vector.wait_ge(w_sem, 48)
                    first = False
                nc.vector.tensor_scalar_add(
                    xp_all[:, s * 4096 + cp * 512: s * 4096 + (cp + 1) * 512],
                    pp[cp % 2][:, :],
                    bias_sb[:, cp:cp + 1],
                ).then_inc(dve_sem, 1)

        nc.sync.drain()
        nc.all_engine_barrier()

    nc.compile()
    return nc


_PROGRAM_CACHE = {}


def _get_program(steps=T):
    if steps not in _PROGRAM_CACHE:
        _PROGRAM_CACHE[steps] = _build_program(steps)
    return _PROGRAM_CACHE[steps]


def _prep_shared(W_ih, W_hh, b_ih, b_hh):
    # lhsT layout [kappa, k*1024 + j] = W[j, k*128+kappa]
    def to_lhsT(W):
        return np.ascontiguousarray(
            W.T.reshape(8, 128, 1024).transpose(1, 0, 2).reshape(128, 8192)
        )

    wih_np = to_lhsT(np.asarray(W_ih)).astype(np.float16)
    whh_np = to_lhsT(np.asarray(W_hh)).astype(np.float16)
    bias_np = np.ascontiguousarray(
        (np.asarray(b_ih) + np.asarray(b_hh)).astype(np.float32).reshape(8, 128).T
    )
    ident_np = np.eye(128, dtype=np.float16)
    return wih_np, whh_np, bias_np, ident_np


TRACE = False
LAST_RESULT = [None]


def kernel(x, W_ih, W_hh, b_ih, b_hh, _steps=T):
    from concourse.bass_utils import run_bass_kernel_spmd

    x = np.asarray(x)
    steps = _steps
    n_slices = steps // WIN
    n_slabs = steps // GY
    nc = _get_program(steps)
    wih_np, whh_np, bias_np, ident_np = _prep_shared(W_ih, W_hh, b_ih, b_hh)

    in_maps = []
    for core in range(N_CORES):
        xs = x[core * BS:(core + 1) * BS, :steps]   # [8, steps, I]
        # xt[kappa, k*(n_slices*512) + t*8 + b] = x[b, t, k*128+kappa]
        xt_np = np.ascontiguousarray(
            xs.transpose(2, 1, 0)                   # [I, steps, B]
            .reshape(8, 128, steps * BS)            # [k, kappa, t*8+b]
            .transpose(1, 0, 2)                     # [kappa, k, t*8+b]
            .reshape(128, 8 * steps * BS)
        ).astype(np.float16)
        in_maps.append({
            "wih": wih_np, "whh": whh_np, "xt": xt_np,
            "ident": ident_np, "bias": bias_np,
        })

    res = run_bass_kernel_spmd(nc, in_maps, list(range(N_CORES)), trace=TRACE)
    LAST_RESULT[0] = res

    out = np.empty((B, T, H), dtype=np.float32)
    for core in range(N_CORES):
        yv = res.results[core]["y"]                 # [n_slabs, 128, GY*64] f16
        hb = (
            yv.reshape(n_slabs, 128, GY, 8, 8)      # [g, kappa, idx, c, b]
            .transpose(4, 0, 2, 3, 1)               # [b, g, idx, c, kappa]
            .reshape(BS, steps, H)
            .astype(np.float32)
        )
        out[core * BS:(core + 1) * BS, :steps] = hb
    return out
